# revision 1
# baseline (speedup 1.0000x reference)
"""Trainium2 kernel for nn_Linear_14912126452257 (scatter_memory).

Computes: new_weight = weight + scatter_add(shira_indices, shira_weight);
          out = x @ new_weight^T + bias

Sharding: column-parallel over out_features across 8 NeuronCores
(each core owns 512 of 4096 output features). x is replicated; the
sparse COO entries are partitioned by owning row-shard.

Per-core device algorithm:
  1. Scatter: entries (r, c, v) of this shard, bucketed by c//128, are
     expanded into one-hot matrices on DVE and accumulated into dense
     delta^T chunks on the PE (one-hot matmul; duplicate indices add
     natively in PSUM).  W'^T[ic] = W^T[ic] + delta^T[ic], cast bf16.
  2. GEMM: out[m, o] = sum_ic xT[ic]^T @ W'^T[ic] in bf16 with fp32
     PSUM accumulation, + bias epilogue on DVE.
Host only marshals data (transpose/cast/bucket/pad) and concatenates
the per-core output shards.
"""

import sys

for _p in ("/opt/trn_rl_repo", "/root/.axon_site/_ro/trn_rl_repo"):
    if _p not in sys.path:
        sys.path.append(_p)

import numpy as np
import ml_dtypes

import concourse.bass as bass
import concourse.mybir as mybir
import concourse.tile as tile
from concourse.bass_utils import run_bass_kernel_spmd

P = 128
IN_F = 4096
OUT_F = 4096
N_CORES = 8
O_SHARD = OUT_F // N_CORES  # 512
NK = IN_F // P  # 32 contraction chunks
M_TOT = 8192  # 4 * 2048 tokens
SUPER_M = 512  # tokens per x super-tile
NSUP = M_TOT // SUPER_M
MT_PER_SUP = SUPER_M // P
SCALING = 1.0


def _build_bass(bucket_tiles):
    """Build the SPMD Bass program. bucket_tiles[ic] = number of 128-entry
    tiles for contraction-chunk bucket ic (same for every core; padded)."""
    t_total = int(sum(bucket_tiles))
    nc = bass.Bass("TRN2", target_bir_lowering=False, debug=False, num_devices=1)

    xt_d = nc.dram_tensor("xt", [IN_F, M_TOT], mybir.dt.bfloat16, kind="ExternalInput").ap()
    wt_d = nc.dram_tensor("wt", [IN_F, O_SHARD], mybir.dt.float32, kind="ExternalInput").ap()
    bias_d = nc.dram_tensor("bias", [P, O_SHARD], mybir.dt.float32, kind="ExternalInput").ap()
    entc_d = nc.dram_tensor("ent_c", [P, t_total], mybir.dt.float32, kind="ExternalInput").ap()
    entr_d = nc.dram_tensor("ent_r", [P, t_total], mybir.dt.float32, kind="ExternalInput").ap()
    entv_d = nc.dram_tensor("ent_v", [P, t_total], mybir.dt.float32, kind="ExternalInput").ap()
    iotao_d = nc.dram_tensor("iota_o", [P, O_SHARD], mybir.dt.float32, kind="ExternalInput").ap()
    iotac_d = nc.dram_tensor("iota_c", [P, P], mybir.dt.float32, kind="ExternalInput").ap()
    out_d = nc.dram_tensor("out", [M_TOT, O_SHARD], mybir.dt.float32, kind="ExternalOutput").ap()

    with tile.TileContext(nc) as tc:
        with (
            tc.tile_pool(name="persist", bufs=1) as persist,
            tc.tile_pool(name="work", bufs=3) as work,
            tc.tile_pool(name="xpool", bufs=2) as xpool,
            tc.tile_pool(name="psum_d", bufs=2, space="PSUM") as psum_d_pool,
            tc.tile_pool(name="psum_o", bufs=4, space="PSUM") as psum_o_pool,
        ):
            wt_bf = persist.tile([P, NK, O_SHARD], mybir.dt.bfloat16)
            iota_o_sb = persist.tile([P, O_SHARD], mybir.dt.float32)
            iota_c_sb = persist.tile([P, P], mybir.dt.float32)
            bias_sb = persist.tile([P, O_SHARD], mybir.dt.float32)
            entc_sb = persist.tile([P, t_total], mybir.dt.float32)
            entr_sb = persist.tile([P, t_total], mybir.dt.float32)
            entv_sb = persist.tile([P, t_total], mybir.dt.float32)
            nc.sync.dma_start(iota_o_sb[:], iotao_d[:])
            nc.sync.dma_start(iota_c_sb[:], iotac_d[:])
            nc.sync.dma_start(bias_sb[:], bias_d[:])
            nc.sync.dma_start(entc_sb[:], entc_d[:])
            nc.sync.dma_start(entr_sb[:], entr_d[:])
            nc.sync.dma_start(entv_sb[:], entv_d[:])

            # ---- scatter: build W'^T (bf16) chunk by chunk ----
            tbase = 0
            for ic in range(NK):
                nt = int(bucket_tiles[ic])
                wtile = work.tile([P, O_SHARD], mybir.dt.float32, tag="wtile")
                nc.sync.dma_start(wtile[:], wt_d[ic * P : (ic + 1) * P, :])
                if nt == 0:
                    nc.vector.tensor_copy(out=wt_bf[:, ic, :], in_=wtile[:])
                    continue
                pd = psum_d_pool.tile([P, O_SHARD], mybir.dt.float32)
                for t in range(nt):
                    col = entc_sb[:, tbase + t : tbase + t + 1]
                    r_ = entr_sb[:, tbase + t : tbase + t + 1]
                    v_ = entv_sb[:, tbase + t : tbase + t + 1]
                    coh = work.tile([P, P], mybir.dt.bfloat16, tag="coh")
                    vcoh = work.tile([P, P], mybir.dt.bfloat16, tag="vcoh")
                    roh = work.tile([P, O_SHARD], mybir.dt.bfloat16, tag="roh")
                    nc.vector.tensor_tensor(
                        out=coh[:], in0=col.to_broadcast([P, P]), in1=iota_c_sb[:],
                        op=mybir.AluOpType.is_equal,
                    )
                    nc.vector.tensor_tensor(
                        out=vcoh[:], in0=coh[:], in1=v_.to_broadcast([P, P]),
                        op=mybir.AluOpType.mult,
                    )
                    nc.vector.tensor_tensor(
                        out=roh[:], in0=r_.to_broadcast([P, O_SHARD]), in1=iota_o_sb[:],
                        op=mybir.AluOpType.is_equal,
                    )
                    nc.tensor.matmul(
                        out=pd[:], lhsT=vcoh[:], rhs=roh[:],
                        start=(t == 0), stop=(t == nt - 1),
                    )
                tbase += nt
                nc.vector.tensor_tensor(
                    out=wt_bf[:, ic, :], in0=wtile[:], in1=pd[:], op=mybir.AluOpType.add
                )

            # ---- GEMM: out[m, o] += xT[ic]^T @ W'^T[ic] ----
            xt_t = xt_d.rearrange("(ko p) m -> p ko m", p=P)  # [P, NK, M_TOT]
            out_t = out_d.rearrange("(mt p) o -> mt p o", p=P)
            for sup in range(NSUP):
                xsb = xpool.tile([P, NK, SUPER_M], mybir.dt.bfloat16, tag="xsb")
                nc.sync.dma_start(
                    xsb[:], xt_t[:, :, sup * SUPER_M : (sup + 1) * SUPER_M]
                )
                for mt in range(MT_PER_SUP):
                    po = psum_o_pool.tile([P, O_SHARD], mybir.dt.float32)
                    for ic in range(NK):
                        nc.tensor.matmul(
                            out=po[:],
                            lhsT=xsb[:, ic, mt * P : (mt + 1) * P],
                            rhs=wt_bf[:, ic, :],
                            start=(ic == 0), stop=(ic == NK - 1),
                        )
                    osb = work.tile([P, O_SHARD], mybir.dt.float32, tag="osb")
                    nc.vector.tensor_tensor(
                        out=osb[:], in0=po[:], in1=bias_sb[:], op=mybir.AluOpType.add
                    )
                    nc.sync.dma_start(out_t[sup * MT_PER_SUP + mt], osb[:])
    return nc


def _split_multi_waits(nc):
    """Walrus in this container rejects compute-engine instructions carrying
    more than one sync wait (setupSyncWait: 'Too many sync wait commands').
    Hoist all-but-none of each such instruction's waits onto standalone
    EventSemaphore (pure wait) instructions inserted just before it in the
    same engine stream — semantically identical, per-engine order preserved."""
    import concourse.mybir as mybir

    n_split = 0
    for fn in nc.m.functions:
        for block in fn.blocks:
            new_instructions = []
            for inst in block.instructions:
                si = getattr(inst, "sync_info", None)
                waits = list(si.on_wait) if si is not None else []
                if len(waits) > 1:
                    for w in waits:
                        n_split += 1
                        new_instructions.append(
                            mybir.InstEventSemaphore(
                                name=f"{inst.name}-w{n_split}",
                                engine=inst.engine,
                                ins=[],
                                outs=[],
                                sync_info=mybir.SyncInfo(
                                    on_wait=[w], on_update=[]
                                ),
                            )
                        )
                    inst.sync_info = mybir.SyncInfo(
                        on_wait=[], on_update=list(si.on_update)
                    )
                new_instructions.append(inst)
            block.instructions = new_instructions
    return n_split


def _prep_inputs(x, weight, bias, shira_weight, shira_indices):
    """Host-side marshalling: transpose/cast x, shard+transpose W, bucket
    and pad the COO entries by (core, c//128)."""
    x2 = np.asarray(x, dtype=np.float32).reshape(M_TOT, IN_F)
    xt = np.ascontiguousarray(x2.T).astype(ml_dtypes.bfloat16)

    w = np.asarray(weight, dtype=np.float32)
    bias_np = np.asarray(bias, dtype=np.float32)
    rows = np.asarray(shira_indices[0]).astype(np.int64)
    cols = np.asarray(shira_indices[1]).astype(np.int64)
    vals = np.asarray(shira_weight, dtype=np.float32) * SCALING

    core = rows // O_SHARD
    r_loc = rows % O_SHARD
    ic = cols // P
    c_lo = cols % P

    # counts[core, ic]
    counts = np.zeros((N_CORES, NK), dtype=np.int64)
    np.add.at(counts, (core, ic), 1)
    bucket_tiles = [int(-(-counts[:, b].max() // P)) for b in range(NK)]
    t_total = int(sum(bucket_tiles))

    # sort entries by (core, ic) for fast segmentation
    order = np.lexsort((ic, core))
    core_s, ic_s = core[order], ic[order]
    r_s, c_s, v_s = r_loc[order], c_lo[order], vals[order]
    # start offset of each (core, ic) segment
    seg_starts = np.searchsorted(core_s * NK + ic_s, np.arange(N_CORES * NK))

    in_maps = []
    iota_o = np.broadcast_to(
        np.arange(O_SHARD, dtype=np.float32), (P, O_SHARD)
    ).copy()
    iota_c = np.broadcast_to(np.arange(P, dtype=np.float32), (P, P)).copy()
    for c in range(N_CORES):
        ec = np.zeros((t_total * P,), np.float32)
        er = np.zeros((t_total * P,), np.float32)
        ev = np.zeros((t_total * P,), np.float32)
        tbase = 0
        for b in range(NK):
            seg = c * NK + b
            s = seg_starts[seg]
            e = seg_starts[seg + 1] if seg + 1 < N_CORES * NK else len(order)
            n = e - s
            off = tbase * P
            ec[off : off + n] = c_s[s:e]
            er[off : off + n] = r_s[s:e]
            ev[off : off + n] = v_s[s:e]
            tbase += bucket_tiles[b]
        # pack [P, T]: entry j of tile t -> [j, t]
        ec = np.ascontiguousarray(ec.reshape(t_total, P).T)
        er = np.ascontiguousarray(er.reshape(t_total, P).T)
        ev = np.ascontiguousarray(ev.reshape(t_total, P).T)
        wt = np.ascontiguousarray(w[c * O_SHARD : (c + 1) * O_SHARD, :].T)
        bias_rep = np.broadcast_to(
            bias_np[c * O_SHARD : (c + 1) * O_SHARD], (P, O_SHARD)
        ).copy()
        in_maps.append(
            {
                "xt": xt,
                "wt": wt,
                "bias": bias_rep,
                "ent_c": ec,
                "ent_r": er,
                "ent_v": ev,
                "iota_o": iota_o,
                "iota_c": iota_c,
            }
        )
    return bucket_tiles, in_maps


def kernel(x, weight, bias, shira_weight, shira_indices, _trace=False):
    bucket_tiles, in_maps = _prep_inputs(
        x, weight, bias, shira_weight, shira_indices
    )
    nc = _build_bass(bucket_tiles)
    _split_multi_waits(nc)
    res = run_bass_kernel_spmd(
        nc, in_maps, core_ids=list(range(N_CORES)), trace=_trace
    )
    out = np.concatenate([r["out"] for r in res.results], axis=1)
    out = out.reshape(4, 2048, OUT_F)
    if _trace:
        kernel.last_results = res
    return out



# revision 2
# speedup vs baseline: 1.3296x; 1.3296x over previous
"""Trainium2 kernel for nn_Linear_14912126452257 (scatter_memory).

Computes: new_weight = weight + scatter_add(shira_indices, shira_weight);
          out = x @ new_weight^T + bias

Sharding: column-parallel over out_features across 8 NeuronCores
(each core owns 512 of 4096 output features). x is replicated; the
sparse COO entries are partitioned by owning row-shard.

Per-core device algorithm:
  1. Scatter: COO entries, bucketed by (c//128, r_loc//128) and padded
     to 128-entry tiles, arrive as host-marshalled one-hot tile pairs
     (vcoh[j, c%128] = v, roh[j, r%128] = 1).  The PE accumulates
     delta^T[c, o] = vcoh^T @ roh into PSUM (duplicates add natively),
     then DVE adds the fp32 weight chunk: W'^T = W^T + delta^T, bf16.
  2. GEMM: out[m, o] = sum_ic xT[ic]^T @ W'^T[ic] in bf16 with fp32
     PSUM accumulation, + bias epilogue on DVE.
Host only marshals data (transpose/cast/bucket/pad/one-hot expand) and
concatenates the per-core output shards.
"""

import sys

for _p in ("/opt/trn_rl_repo", "/root/.axon_site/_ro/trn_rl_repo"):
    if _p not in sys.path:
        sys.path.append(_p)

import numpy as np
import ml_dtypes

import concourse.bass as bass
import concourse.mybir as mybir
import concourse.tile as tile
from concourse.bass_utils import run_bass_kernel_spmd

P = 128
IN_F = 4096
OUT_F = 4096
N_CORES = 8
O_SHARD = OUT_F // N_CORES  # 512
NK = IN_F // P  # 32 contraction chunks
NOC = O_SHARD // P  # 4 output sub-chunks per core
M_TOT = 8192  # 4 * 2048 tokens
SUPER_M = 512  # tokens per x super-tile
NSUP = M_TOT // SUPER_M
MT_PER_SUP = SUPER_M // P
SCALING = 1.0


def _build_bass(bucket_tiles):
    """Build the SPMD Bass program. bucket_tiles[ic][oc] = number of
    128-entry one-hot tile pairs for bucket (ic, oc); same for every
    core (padded)."""
    ic_tiles = [int(sum(bucket_tiles[ic])) for ic in range(NK)]
    t_total = int(sum(ic_tiles))
    nc = bass.Bass("TRN2", target_bir_lowering=False, debug=False, num_devices=1)

    xt_d = nc.dram_tensor("xt", [IN_F, M_TOT], mybir.dt.bfloat16, kind="ExternalInput").ap()
    wt_d = nc.dram_tensor("wt", [IN_F, O_SHARD], mybir.dt.float32, kind="ExternalInput").ap()
    bias_d = nc.dram_tensor("bias", [P, O_SHARD], mybir.dt.float32, kind="ExternalInput").ap()
    # one-hot tile pairs: [:, t, 0:128] = vcoh (values), [:, t, 128:256] = roh
    oh_d = nc.dram_tensor("oh", [P, t_total, 2 * P], mybir.dt.bfloat16, kind="ExternalInput").ap()
    out_d = nc.dram_tensor("out", [M_TOT, O_SHARD], mybir.dt.float32, kind="ExternalOutput").ap()

    with tile.TileContext(nc) as tc:
        with (
            tc.tile_pool(name="persist", bufs=1) as persist,
            tc.tile_pool(name="work", bufs=3) as work,
            tc.tile_pool(name="ohpool", bufs=2) as ohpool,
            tc.tile_pool(name="xpool", bufs=2) as xpool,
            tc.tile_pool(name="psum_d", bufs=2, space="PSUM") as psum_d_pool,
            tc.tile_pool(name="psum_o", bufs=4, space="PSUM") as psum_o_pool,
        ):
            wt_bf = persist.tile([P, NK, O_SHARD], mybir.dt.bfloat16)
            bias_sb = persist.tile([P, O_SHARD], mybir.dt.float32)
            nc.sync.dma_start(bias_sb[:], bias_d[:])

            # ---- scatter: build W'^T (bf16) chunk by chunk ----
            tbase = 0
            for ic in range(NK):
                nt = ic_tiles[ic]
                ohc = ohpool.tile([P, nt, 2 * P], mybir.dt.bfloat16, tag="ohc")
                nc.sync.dma_start(ohc[:], oh_d[:, tbase : tbase + nt, :])
                wtile = work.tile([P, O_SHARD], mybir.dt.float32, tag="wtile")
                nc.sync.dma_start(wtile[:], wt_d[ic * P : (ic + 1) * P, :])
                pd = psum_d_pool.tile([P, O_SHARD], mybir.dt.float32)
                t = 0
                for oc in range(NOC):
                    ntoc = int(bucket_tiles[ic][oc])
                    for j in range(ntoc):
                        nc.tensor.matmul(
                            out=pd[:, oc * P : (oc + 1) * P],
                            lhsT=ohc[:, t, 0:P],
                            rhs=ohc[:, t, P : 2 * P],
                            start=(j == 0), stop=(j == ntoc - 1),
                        )
                        t += 1
                tbase += nt
                nc.vector.tensor_tensor(
                    out=wt_bf[:, ic, :], in0=wtile[:], in1=pd[:], op=mybir.AluOpType.add
                )

            # ---- GEMM: out[m, o] += xT[ic]^T @ W'^T[ic] ----
            xt_t = xt_d.rearrange("(ko p) m -> p ko m", p=P)  # [P, NK, M_TOT]
            out_t = out_d.rearrange("(mt p) o -> mt p o", p=P)
            for sup in range(NSUP):
                xsb = xpool.tile([P, NK, SUPER_M], mybir.dt.bfloat16, tag="xsb")
                nc.sync.dma_start(
                    xsb[:], xt_t[:, :, sup * SUPER_M : (sup + 1) * SUPER_M]
                )
                for mt in range(MT_PER_SUP):
                    po = psum_o_pool.tile([P, O_SHARD], mybir.dt.float32)
                    for ic in range(NK):
                        nc.tensor.matmul(
                            out=po[:],
                            lhsT=xsb[:, ic, mt * P : (mt + 1) * P],
                            rhs=wt_bf[:, ic, :],
                            start=(ic == 0), stop=(ic == NK - 1),
                        )
                    osb = work.tile([P, O_SHARD], mybir.dt.float32, tag="osb")
                    nc.vector.tensor_tensor(
                        out=osb[:], in0=po[:], in1=bias_sb[:], op=mybir.AluOpType.add
                    )
                    nc.sync.dma_start(out_t[sup * MT_PER_SUP + mt], osb[:])
    return nc


def _split_multi_waits(nc):
    """Walrus in this container rejects compute-engine instructions carrying
    more than one sync wait (setupSyncWait: 'Too many sync wait commands').
    Hoist all-but-none of each such instruction's waits onto standalone
    EventSemaphore (pure wait) instructions inserted just before it in the
    same engine stream — semantically identical, per-engine order preserved."""
    import concourse.mybir as mybir

    n_split = 0
    for fn in nc.m.functions:
        for block in fn.blocks:
            new_instructions = []
            for inst in block.instructions:
                si = getattr(inst, "sync_info", None)
                waits = list(si.on_wait) if si is not None else []
                if len(waits) > 1:
                    for w in waits:
                        n_split += 1
                        new_instructions.append(
                            mybir.InstEventSemaphore(
                                name=f"{inst.name}-w{n_split}",
                                engine=inst.engine,
                                ins=[],
                                outs=[],
                                sync_info=mybir.SyncInfo(
                                    on_wait=[w], on_update=[]
                                ),
                            )
                        )
                    inst.sync_info = mybir.SyncInfo(
                        on_wait=[], on_update=list(si.on_update)
                    )
                new_instructions.append(inst)
            block.instructions = new_instructions
    return n_split


def _prep_inputs(x, weight, bias, shira_weight, shira_indices):
    """Host-side marshalling: transpose/cast x, shard+transpose W, bucket
    the COO entries by (core, c//128, r_loc//128), pad to 128-entry tiles
    and expand into one-hot tile pairs."""
    x2 = np.asarray(x, dtype=np.float32).reshape(M_TOT, IN_F)
    xt = np.ascontiguousarray(x2.T).astype(ml_dtypes.bfloat16)

    w = np.asarray(weight, dtype=np.float32)
    bias_np = np.asarray(bias, dtype=np.float32)
    rows = np.asarray(shira_indices[0]).astype(np.int64)
    cols = np.asarray(shira_indices[1]).astype(np.int64)
    vals = (np.asarray(shira_weight, dtype=np.float32) * SCALING).astype(
        ml_dtypes.bfloat16
    )

    core = rows // O_SHARD
    r_loc = rows % O_SHARD
    oc = r_loc // P
    r128 = r_loc % P
    ic = cols // P
    c128 = cols % P

    # bucket = (core, ic, oc); counts per bucket
    NB = NK * NOC
    bucket = ic * NOC + oc  # 0..127 within a core
    counts = np.zeros((N_CORES, NB), dtype=np.int64)
    np.add.at(counts, (core, bucket), 1)
    # padded tiles per bucket: max across cores, at least 1
    bt_flat = np.maximum(1, -(-counts.max(axis=0) // P))  # [NB]
    bucket_tiles = bt_flat.reshape(NK, NOC)
    t_total = int(bt_flat.sum())
    tile_base = np.concatenate([[0], np.cumsum(bt_flat)[:-1]])  # [NB]

    # sort entries by (core, bucket); rank within segment
    key = core * NB + bucket
    order = np.argsort(key, kind="stable")
    key_s = key[order]
    seg_starts = np.searchsorted(key_s, np.arange(N_CORES * NB))
    rank = np.arange(len(order)) - np.repeat(
        seg_starts, np.diff(np.concatenate([seg_starts, [len(order)]]))
    )
    core_s = core[order]
    b_s = bucket[order]
    # global entry slot: tile t = tile_base[b] + rank//P, row p = rank%P
    t_idx = tile_base[b_s] + rank // P
    p_idx = rank % P

    in_maps = []
    for c in range(N_CORES):
        m = core_s == c
        # oh[p, t, 0:128] = vcoh (value one-hot over c128)
        # oh[p, t, 128:256] = roh (one-hot over r128)
        oh = np.zeros((P, t_total, 2 * P), dtype=ml_dtypes.bfloat16)
        oh[p_idx[m], t_idx[m], c128[order][m]] = vals[order][m]
        oh[p_idx[m], t_idx[m], P + r128[order][m]] = 1.0
        wt = np.ascontiguousarray(w[c * O_SHARD : (c + 1) * O_SHARD, :].T)
        bias_rep = np.broadcast_to(
            bias_np[c * O_SHARD : (c + 1) * O_SHARD], (P, O_SHARD)
        ).copy()
        in_maps.append({"xt": xt, "wt": wt, "bias": bias_rep, "oh": oh})
    return bucket_tiles, in_maps


def kernel(x, weight, bias, shira_weight, shira_indices, _trace=False):
    bucket_tiles, in_maps = _prep_inputs(
        x, weight, bias, shira_weight, shira_indices
    )
    nc = _build_bass(bucket_tiles)
    _split_multi_waits(nc)
    res = run_bass_kernel_spmd(
        nc, in_maps, core_ids=list(range(N_CORES)), trace=_trace
    )
    out = np.concatenate([r["out"] for r in res.results], axis=1)
    out = out.reshape(4, 2048, OUT_F)
    if _trace:
        kernel.last_results = res
    return out


# revision 6
# speedup vs baseline: 1.3574x; 1.0209x over previous
"""Trainium2 kernel for nn_Linear_14912126452257 (scatter_memory).

Computes: new_weight = weight + scatter_add(shira_indices, shira_weight);
          out = x @ new_weight^T + bias

Sharding: column-parallel over out_features across 8 NeuronCores
(each core owns 512 of 4096 output features). x is replicated; the
sparse COO entries are partitioned by owning row-shard.

Per-core device algorithm:
  1. Scatter: COO entries, bucketed by (c//128, r_loc//128) and padded
     to 128-entry tiles, arrive as host-marshalled one-hot tile pairs in
     fp8-e3m4 (vcoh[j, c%128] = v*64, roh[j, r%128] = 1).  The PE
     accumulates delta^T[c, o] = vcoh^T @ roh into PSUM (duplicates add
     natively); DVE then fuses descale+add: W'^T = pd/64 + W^T, bf16.
  2. GEMM: out[m, o] = sum_ic xT[ic]^T @ W'^T[ic] in bf16 with fp32
     PSUM accumulation, + bias epilogue on DVE.  The first six GEMM
     m-tiles are interleaved into the scatter loop (accumulating chunk
     by chunk as W' chunks appear) so the PE stays busy while the
     one-hot tiles stream in from HBM.
Host only marshals data (transpose/cast/bucket/pad/one-hot expand) and
concatenates the per-core output shards.
"""

import sys

for _p in ("/opt/trn_rl_repo", "/root/.axon_site/_ro/trn_rl_repo"):
    if _p not in sys.path:
        sys.path.append(_p)

import numpy as np
import ml_dtypes

import concourse.bass as bass
import concourse.mybir as mybir
import concourse.tile as tile
from concourse.bass_utils import run_bass_kernel_spmd

P = 128
IN_F = 4096
OUT_F = 4096
N_CORES = 8
O_SHARD = OUT_F // N_CORES  # 512
NK = IN_F // P  # 32 contraction chunks
NOC = O_SHARD // P  # 4 output sub-chunks per core
M_TOT = 8192  # 4 * 2048 tokens
SUPER_M = 512  # tokens per x super-tile
NSUP = M_TOT // SUPER_M
MT_PER_SUP = SUPER_M // P
SCALING = 1.0
VSCALE = 64.0  # fp8-e3m4 value pre-scale (min normal 2^-2; v ~ 0.02)
N_EARLY = 6  # GEMM m-tiles interleaved into the scatter loop


def _build_bass(bucket_tiles):
    """Build the SPMD Bass program. bucket_tiles[ic][oc] = number of
    128-entry one-hot tile pairs for bucket (ic, oc); same for every
    core (padded)."""
    ic_tiles = [int(sum(bucket_tiles[ic])) for ic in range(NK)]
    t_total = int(sum(ic_tiles))
    nc = bass.Bass("TRN2", target_bir_lowering=False, debug=False, num_devices=1)

    xt_d = nc.dram_tensor("xt", [IN_F, M_TOT], mybir.dt.bfloat16, kind="ExternalInput").ap()
    wt_d = nc.dram_tensor("wt", [IN_F, O_SHARD], mybir.dt.bfloat16, kind="ExternalInput").ap()
    bias_d = nc.dram_tensor("bias", [P, O_SHARD], mybir.dt.float32, kind="ExternalInput").ap()
    # one-hot tile pairs: [:, t, 0:128] = vcoh (values*64), [:, t, 128:256] = roh
    oh_d = nc.dram_tensor("oh", [P, t_total, 2 * P], mybir.dt.float8e3, kind="ExternalInput").ap()
    out_d = nc.dram_tensor("out", [M_TOT, O_SHARD], mybir.dt.float32, kind="ExternalOutput").ap()

    xt_t = xt_d.rearrange("(ko p) m -> p ko m", p=P)  # [P, NK, M_TOT]
    out_t = out_d.rearrange("(mt p) o -> mt p o", p=P)

    with tile.TileContext(nc) as tc:
        with (
            tc.tile_pool(name="persist", bufs=1) as persist,
            tc.tile_pool(name="work", bufs=3) as work,
            tc.tile_pool(name="ohpool", bufs=2) as ohpool,
            tc.tile_pool(name="xpool", bufs=3) as xpool,
        ):
            wt_bf = persist.tile([P, NK, O_SHARD], mybir.dt.bfloat16)
            bias_sb = persist.tile([P, O_SHARD], mybir.dt.float32)
            nc.sync.dma_start(bias_sb[:], bias_d[:])

            def load_sup(sup):
                xsb = xpool.tile([P, NK, SUPER_M], mybir.dt.bfloat16, tag="xsb")
                nc.sync.dma_start(
                    xsb[:], xt_t[:, :, sup * SUPER_M : (sup + 1) * SUPER_M]
                )
                return xsb

            def epilogue(po, sup, mt):
                osb = work.tile([P, O_SHARD], mybir.dt.float32, tag="osb")
                nc.vector.tensor_tensor(
                    out=osb[:], in0=po[:], in1=bias_sb[:], op=mybir.AluOpType.add
                )
                nc.sync.dma_start(out_t[sup * MT_PER_SUP + mt], osb[:])

            xsb0 = load_sup(0)
            xsb1 = load_sup(1)
            early_src = [
                (xsb0, 0, 0), (xsb0, 0, 1), (xsb0, 0, 2), (xsb0, 0, 3),
                (xsb1, 1, 0), (xsb1, 1, 1),
            ][:N_EARLY]

            # ---- opening: scatter W' chunks, with early GEMM interleaved ----
            with (
                tc.tile_pool(name="psum_d", bufs=2, space="PSUM") as psum_d_pool,
                tc.tile_pool(name="psum_e", bufs=1, space="PSUM") as psum_e_pool,
            ):
                early = [
                    psum_e_pool.tile([P, O_SHARD], mybir.dt.float32, name=f"early{k}", tag=f"early{k}")
                    for k in range(N_EARLY)
                ]
                tbase = 0
                for ic in range(NK):
                    nt = ic_tiles[ic]
                    ohc = ohpool.tile([P, nt, 2 * P], mybir.dt.float8e3, tag="ohc")
                    nc.sync.dma_start(ohc[:], oh_d[:, tbase : tbase + nt, :])
                    wtile = work.tile([P, O_SHARD], mybir.dt.bfloat16, tag="wtile")
                    nc.sync.dma_start(wtile[:], wt_d[ic * P : (ic + 1) * P, :])
                    pd = psum_d_pool.tile([P, O_SHARD], mybir.dt.float32)
                    t = 0
                    for oc in range(NOC):
                        ntoc = int(bucket_tiles[ic][oc])
                        for j in range(ntoc):
                            nc.tensor.matmul(
                                out=pd[:, oc * P : (oc + 1) * P],
                                lhsT=ohc[:, t, 0:P],
                                rhs=ohc[:, t, P : 2 * P],
                                start=(j == 0), stop=(j == ntoc - 1),
                            )
                            t += 1
                    tbase += nt
                    # W'^T chunk = pd/VSCALE + W^T chunk, cast bf16
                    nc.vector.scalar_tensor_tensor(
                        out=wt_bf[:, ic, :],
                        in0=pd[:],
                        scalar=1.0 / VSCALE,
                        in1=wtile[:],
                        op0=mybir.AluOpType.mult,
                        op1=mybir.AluOpType.add,
                    )
                    for k, (xsb, _, mt) in enumerate(early_src):
                        nc.tensor.matmul(
                            out=early[k][:],
                            lhsT=xsb[:, ic, mt * P : (mt + 1) * P],
                            rhs=wt_bf[:, ic, :],
                            start=(ic == 0), stop=(ic == NK - 1),
                        )
                for k, (_, sup, mt) in enumerate(early_src):
                    epilogue(early[k], sup, mt)

            # ---- main GEMM: remaining m-tiles ----
            with tc.tile_pool(name="psum_o", bufs=4, space="PSUM") as psum_o_pool:
                def gemm_tile(xsb, sup, mt):
                    po = psum_o_pool.tile([P, O_SHARD], mybir.dt.float32)
                    for ic in range(NK):
                        nc.tensor.matmul(
                            out=po[:],
                            lhsT=xsb[:, ic, mt * P : (mt + 1) * P],
                            rhs=wt_bf[:, ic, :],
                            start=(ic == 0), stop=(ic == NK - 1),
                        )
                    epilogue(po, sup, mt)

                done = {(s, m) for (_, s, m) in early_src}
                for mt in range(MT_PER_SUP):
                    if (1, mt) not in done:
                        gemm_tile(xsb1, 1, mt)
                for sup in range(2, NSUP):
                    xsb = load_sup(sup)
                    for mt in range(MT_PER_SUP):
                        gemm_tile(xsb, sup, mt)
    return nc


def _split_multi_waits(nc):
    """Walrus in this container rejects compute-engine instructions carrying
    more than one sync wait (setupSyncWait: 'Too many sync wait commands').
    Hoist all-but-none of each such instruction's waits onto standalone
    EventSemaphore (pure wait) instructions inserted just before it in the
    same engine stream — semantically identical, per-engine order preserved."""
    import concourse.mybir as mybir

    n_split = 0
    for fn in nc.m.functions:
        for block in fn.blocks:
            new_instructions = []
            for inst in block.instructions:
                si = getattr(inst, "sync_info", None)
                waits = list(si.on_wait) if si is not None else []
                if len(waits) > 1:
                    for w in waits:
                        n_split += 1
                        new_instructions.append(
                            mybir.InstEventSemaphore(
                                name=f"{inst.name}-w{n_split}",
                                engine=inst.engine,
                                ins=[],
                                outs=[],
                                sync_info=mybir.SyncInfo(
                                    on_wait=[w], on_update=[]
                                ),
                            )
                        )
                    inst.sync_info = mybir.SyncInfo(
                        on_wait=[], on_update=list(si.on_update)
                    )
                new_instructions.append(inst)
            block.instructions = new_instructions
    return n_split


def _prep_inputs(x, weight, bias, shira_weight, shira_indices):
    """Host-side marshalling: transpose/cast x, shard+transpose W, bucket
    the COO entries by (core, c//128, r_loc//128), pad to 128-entry tiles
    and expand into fp8 one-hot tile pairs."""
    x2 = np.asarray(x, dtype=np.float32).reshape(M_TOT, IN_F)
    xt = np.ascontiguousarray(x2.T).astype(ml_dtypes.bfloat16)

    w = np.asarray(weight, dtype=np.float32)
    bias_np = np.asarray(bias, dtype=np.float32)
    rows = np.asarray(shira_indices[0]).astype(np.int64)
    cols = np.asarray(shira_indices[1]).astype(np.int64)
    vals = (np.asarray(shira_weight, dtype=np.float32) * SCALING * VSCALE).astype(
        ml_dtypes.float8_e3m4
    )

    core = rows // O_SHARD
    r_loc = rows % O_SHARD
    oc = r_loc // P
    r128 = r_loc % P
    ic = cols // P
    c128 = cols % P

    # bucket = (core, ic, oc); counts per bucket
    NB = NK * NOC
    bucket = ic * NOC + oc  # 0..127 within a core
    counts = np.zeros((N_CORES, NB), dtype=np.int64)
    np.add.at(counts, (core, bucket), 1)
    # padded tiles per bucket: max across cores, at least 1
    bt_flat = np.maximum(1, -(-counts.max(axis=0) // P))  # [NB]
    bucket_tiles = bt_flat.reshape(NK, NOC)
    t_total = int(bt_flat.sum())
    tile_base = np.concatenate([[0], np.cumsum(bt_flat)[:-1]])  # [NB]

    # sort entries by (core, bucket); rank within segment
    key = core * NB + bucket
    order = np.argsort(key, kind="stable")
    key_s = key[order]
    seg_starts = np.searchsorted(key_s, np.arange(N_CORES * NB))
    rank = np.arange(len(order)) - np.repeat(
        seg_starts, np.diff(np.concatenate([seg_starts, [len(order)]]))
    )
    core_s = core[order]
    b_s = bucket[order]
    # global entry slot: tile t = tile_base[b] + rank//P, row p = rank%P
    t_idx = tile_base[b_s] + rank // P
    p_idx = rank % P

    in_maps = []
    for c in range(N_CORES):
        m = core_s == c
        # oh[p, t, 0:128] = vcoh (value one-hot over c128, pre-scaled)
        # oh[p, t, 128:256] = roh (one-hot over r128)
        oh = np.zeros((P, t_total, 2 * P), dtype=ml_dtypes.float8_e3m4)
        oh[p_idx[m], t_idx[m], c128[order][m]] = vals[order][m]
        oh[p_idx[m], t_idx[m], P + r128[order][m]] = 1.0
        wt = np.ascontiguousarray(
            w[c * O_SHARD : (c + 1) * O_SHARD, :].T
        ).astype(ml_dtypes.bfloat16)
        bias_rep = np.broadcast_to(
            bias_np[c * O_SHARD : (c + 1) * O_SHARD], (P, O_SHARD)
        ).copy()
        in_maps.append({"xt": xt, "wt": wt, "bias": bias_rep, "oh": oh})
    return bucket_tiles, in_maps


def kernel(x, weight, bias, shira_weight, shira_indices, _trace=False):
    bucket_tiles, in_maps = _prep_inputs(
        x, weight, bias, shira_weight, shira_indices
    )
    nc = _build_bass(bucket_tiles)
    _split_multi_waits(nc)
    res = run_bass_kernel_spmd(
        nc, in_maps, core_ids=list(range(N_CORES)), trace=_trace
    )
    out = np.concatenate([r["out"] for r in res.results], axis=1)
    out = out.reshape(4, 2048, OUT_F)
    if _trace:
        kernel.last_results = res
    return out


# revision 9
# speedup vs baseline: 1.4590x; 1.0748x over previous
"""Trainium2 kernel for nn_Linear_14912126452257 (scatter_memory).

Computes: new_weight = weight + scatter_add(shira_indices, shira_weight);
          out = x @ new_weight^T + bias

Sharding: column-parallel over out_features across 8 NeuronCores
(each core owns 512 of 4096 output features). x is replicated; the
sparse COO entries are partitioned by owning row-shard.

Per-core device algorithm:
  1. Scatter: COO entries, bucketed by (c//128, r_loc//128) and padded
     to 128-entry tiles, arrive as host-marshalled one-hot tile pairs in
     fp8-e3m4 (vcoh[j, c%128] = v*64, roh[j, r%128] = 1).  The PE
     accumulates delta^T[c, o] = vcoh^T @ roh into PSUM (duplicates add
     natively); DVE then fuses descale+add: W'^T = pd/64 + W^T, bf16.
  2. GEMM: out[m, o] = sum_ic xT[ic]^T @ W'^T[ic] in bf16 with fp32
     PSUM accumulation, + bias epilogue on DVE.  The first six GEMM
     m-tiles are interleaved into the scatter loop (accumulating chunk
     by chunk as W' chunks appear) so the PE stays busy while the
     one-hot tiles stream in from HBM.
Host only marshals data (transpose/cast/bucket/pad/one-hot expand) and
concatenates the per-core output shards.
"""

import sys

for _p in ("/opt/trn_rl_repo", "/root/.axon_site/_ro/trn_rl_repo"):
    if _p not in sys.path:
        sys.path.append(_p)

import numpy as np
import ml_dtypes

import concourse.bass as bass
import concourse.mybir as mybir
import concourse.tile as tile
from concourse.bass_utils import run_bass_kernel_spmd

P = 128
IN_F = 4096
OUT_F = 4096
N_CORES = 8
O_SHARD = OUT_F // N_CORES  # 512
NK = IN_F // P  # 32 contraction chunks
NOC = O_SHARD // P  # 4 output sub-chunks per core
M_TOT = 8192  # 4 * 2048 tokens
SUPER_M = 512  # tokens per x super-tile
NSUP = M_TOT // SUPER_M
MT_PER_SUP = SUPER_M // P
SCALING = 1.0
VSCALE = 64.0  # fp8-e3m4 value pre-scale (min normal 2^-2; v ~ 0.02)
N_EARLY = 6  # GEMM m-tiles interleaved into the scatter loop


def _build_bass(bucket_tiles):
    """Build the SPMD Bass program. bucket_tiles[ic][oc] = number of
    128-entry one-hot tile pairs for bucket (ic, oc); same for every
    core (padded)."""
    ic_tiles = [int(sum(bucket_tiles[ic])) for ic in range(NK)]
    t_total = int(sum(ic_tiles))
    nc = bass.Bass("TRN2", target_bir_lowering=False, debug=False, num_devices=1)

    xt_d = nc.dram_tensor("xt", [IN_F, M_TOT], mybir.dt.bfloat16, kind="ExternalInput").ap()
    wt_d = nc.dram_tensor("wt", [IN_F, O_SHARD], mybir.dt.bfloat16, kind="ExternalInput").ap()
    bias_d = nc.dram_tensor("bias", [P, O_SHARD], mybir.dt.float32, kind="ExternalInput").ap()
    # one-hot tile pairs: [:, t, 0:128] = vcoh (values*64), [:, t, 128:256] = roh
    oh_d = nc.dram_tensor("oh", [P, t_total, 2 * P], mybir.dt.float8e3, kind="ExternalInput").ap()
    out_d = nc.dram_tensor("out", [M_TOT, O_SHARD], mybir.dt.float32, kind="ExternalOutput").ap()

    xt_t = xt_d.rearrange("(ko p) m -> p ko m", p=P)  # [P, NK, M_TOT]
    out_t = out_d.rearrange("(mt p) o -> mt p o", p=P)

    with tile.TileContext(nc) as tc:
        with (
            tc.tile_pool(name="persist", bufs=1) as persist,
            tc.tile_pool(name="work", bufs=3) as work,
            tc.tile_pool(name="ohpool", bufs=2) as ohpool,
            tc.tile_pool(name="xpool", bufs=3) as xpool,
        ):
            wt_bf = persist.tile([P, NK, O_SHARD], mybir.dt.bfloat16)
            bias_sb = persist.tile([P, O_SHARD], mybir.dt.float32)
            nc.sync.dma_start(bias_sb[:], bias_d[:])

            def load_sup(sup, defer=False):
                xsb = xpool.tile([P, NK, SUPER_M], mybir.dt.bfloat16, tag="xsb")
                if not defer:
                    nc.sync.dma_start(
                        xsb[:], xt_t[:, :, sup * SUPER_M : (sup + 1) * SUPER_M]
                    )
                return xsb

            def load_sup_chunk(xsb, sup, ic):
                nc.sync.dma_start(
                    xsb[:, ic, :],
                    xt_t[:, ic, sup * SUPER_M : (sup + 1) * SUPER_M],
                )

            def epilogue(po, sup, mt):
                osb = work.tile([P, O_SHARD], mybir.dt.float32, tag="osb")
                nc.vector.tensor_tensor(
                    out=osb[:], in0=po[:], in1=bias_sb[:], op=mybir.AluOpType.add
                )
                nc.sync.dma_start(out_t[sup * MT_PER_SUP + mt], osb[:])

            xsb0 = load_sup(0, defer=True)
            xsb1 = load_sup(1, defer=True)
            early_src = [
                (xsb0, 0, 0), (xsb0, 0, 1), (xsb0, 0, 2), (xsb0, 0, 3),
                (xsb1, 1, 0), (xsb1, 1, 1),
            ][:N_EARLY]

            # ---- opening: scatter W' chunks, with early GEMM interleaved ----
            with (
                tc.tile_pool(name="psum_d", bufs=2, space="PSUM") as psum_d_pool,
                tc.tile_pool(name="psum_e", bufs=1, space="PSUM") as psum_e_pool,
            ):
                early = [
                    psum_e_pool.tile([P, O_SHARD], mybir.dt.float32, name=f"early{k}", tag=f"early{k}")
                    for k in range(N_EARLY)
                ]
                def early_mms(ic):
                    # early GEMM matmuls for chunk ic (lagged one chunk so
                    # the DVE chunk-assembly overlaps PE work)
                    for k, (xsb, _, mt) in enumerate(early_src):
                        nc.tensor.matmul(
                            out=early[k][:],
                            lhsT=xsb[:, ic, mt * P : (mt + 1) * P],
                            rhs=wt_bf[:, ic, :],
                            start=(ic == 0), stop=(ic == NK - 1),
                        )

                tbase = 0
                for ic in range(NK):
                    nt = ic_tiles[ic]
                    ohc = ohpool.tile([P, nt, 2 * P], mybir.dt.float8e3, tag="ohc")
                    nc.sync.dma_start(ohc[:], oh_d[:, tbase : tbase + nt, :])
                    wtile = work.tile([P, O_SHARD], mybir.dt.bfloat16, tag="wtile")
                    nc.sync.dma_start(wtile[:], wt_d[ic * P : (ic + 1) * P, :])
                    load_sup_chunk(xsb0, 0, ic)
                    load_sup_chunk(xsb1, 1, ic)
                    pd = psum_d_pool.tile([P, O_SHARD], mybir.dt.float32)
                    t = 0
                    for oc in range(NOC):
                        ntoc = int(bucket_tiles[ic][oc])
                        for j in range(ntoc):
                            nc.tensor.matmul(
                                out=pd[:, oc * P : (oc + 1) * P],
                                lhsT=ohc[:, t, 0:P],
                                rhs=ohc[:, t, P : 2 * P],
                                start=(j == 0), stop=(j == ntoc - 1),
                            )
                            t += 1
                    tbase += nt
                    # W'^T chunk = pd/VSCALE + W^T chunk, cast bf16
                    nc.vector.scalar_tensor_tensor(
                        out=wt_bf[:, ic, :],
                        in0=pd[:],
                        scalar=1.0 / VSCALE,
                        in1=wtile[:],
                        op0=mybir.AluOpType.mult,
                        op1=mybir.AluOpType.add,
                    )
                    if ic >= 1:
                        early_mms(ic - 1)
                early_mms(NK - 1)
                for k, (_, sup, mt) in enumerate(early_src):
                    epilogue(early[k], sup, mt)

            # ---- main GEMM: remaining m-tiles ----
            with tc.tile_pool(name="psum_o", bufs=4, space="PSUM") as psum_o_pool:
                def gemm_tile(xsb, sup, mt):
                    po = psum_o_pool.tile([P, O_SHARD], mybir.dt.float32)
                    for ic in range(NK):
                        nc.tensor.matmul(
                            out=po[:],
                            lhsT=xsb[:, ic, mt * P : (mt + 1) * P],
                            rhs=wt_bf[:, ic, :],
                            start=(ic == 0), stop=(ic == NK - 1),
                        )
                    epilogue(po, sup, mt)

                done = {(s, m) for (_, s, m) in early_src}
                for mt in range(MT_PER_SUP):
                    if (1, mt) not in done:
                        gemm_tile(xsb1, 1, mt)
                for sup in range(2, NSUP):
                    xsb = load_sup(sup)
                    for mt in range(MT_PER_SUP):
                        gemm_tile(xsb, sup, mt)
    return nc


def _split_multi_waits(nc):
    """Walrus in this container rejects compute-engine instructions carrying
    more than one sync wait (setupSyncWait: 'Too many sync wait commands').
    Hoist all-but-none of each such instruction's waits onto standalone
    EventSemaphore (pure wait) instructions inserted just before it in the
    same engine stream — semantically identical, per-engine order preserved."""
    import concourse.mybir as mybir

    n_split = 0
    for fn in nc.m.functions:
        for block in fn.blocks:
            new_instructions = []
            for inst in block.instructions:
                si = getattr(inst, "sync_info", None)
                waits = list(si.on_wait) if si is not None else []
                if len(waits) > 1:
                    for w in waits:
                        n_split += 1
                        new_instructions.append(
                            mybir.InstEventSemaphore(
                                name=f"{inst.name}-w{n_split}",
                                engine=inst.engine,
                                ins=[],
                                outs=[],
                                sync_info=mybir.SyncInfo(
                                    on_wait=[w], on_update=[]
                                ),
                            )
                        )
                    inst.sync_info = mybir.SyncInfo(
                        on_wait=[], on_update=list(si.on_update)
                    )
                new_instructions.append(inst)
            block.instructions = new_instructions
    return n_split


def _prep_inputs(x, weight, bias, shira_weight, shira_indices):
    """Host-side marshalling: transpose/cast x, shard+transpose W, bucket
    the COO entries by (core, c//128, r_loc//128), pad to 128-entry tiles
    and expand into fp8 one-hot tile pairs."""
    x2 = np.asarray(x, dtype=np.float32).reshape(M_TOT, IN_F)
    xt = np.ascontiguousarray(x2.T).astype(ml_dtypes.bfloat16)

    w = np.asarray(weight, dtype=np.float32)
    bias_np = np.asarray(bias, dtype=np.float32)
    rows = np.asarray(shira_indices[0]).astype(np.int64)
    cols = np.asarray(shira_indices[1]).astype(np.int64)
    vals = (np.asarray(shira_weight, dtype=np.float32) * SCALING * VSCALE).astype(
        ml_dtypes.float8_e3m4
    )

    core = rows // O_SHARD
    r_loc = rows % O_SHARD
    oc = r_loc // P
    r128 = r_loc % P
    ic = cols // P
    c128 = cols % P

    # bucket = (core, ic, oc); counts per bucket
    NB = NK * NOC
    bucket = ic * NOC + oc  # 0..127 within a core
    counts = np.zeros((N_CORES, NB), dtype=np.int64)
    np.add.at(counts, (core, bucket), 1)
    # padded tiles per bucket: max across cores, at least 1
    bt_flat = np.maximum(1, -(-counts.max(axis=0) // P))  # [NB]
    bucket_tiles = bt_flat.reshape(NK, NOC)
    t_total = int(bt_flat.sum())
    tile_base = np.concatenate([[0], np.cumsum(bt_flat)[:-1]])  # [NB]

    # sort entries by (core, bucket); rank within segment
    key = core * NB + bucket
    order = np.argsort(key, kind="stable")
    key_s = key[order]
    seg_starts = np.searchsorted(key_s, np.arange(N_CORES * NB))
    rank = np.arange(len(order)) - np.repeat(
        seg_starts, np.diff(np.concatenate([seg_starts, [len(order)]]))
    )
    core_s = core[order]
    b_s = bucket[order]
    # global entry slot: tile t = tile_base[b] + rank//P, row p = rank%P
    t_idx = tile_base[b_s] + rank // P
    p_idx = rank % P

    in_maps = []
    for c in range(N_CORES):
        m = core_s == c
        # oh[p, t, 0:128] = vcoh (value one-hot over c128, pre-scaled)
        # oh[p, t, 128:256] = roh (one-hot over r128)
        oh = np.zeros((P, t_total, 2 * P), dtype=ml_dtypes.float8_e3m4)
        oh[p_idx[m], t_idx[m], c128[order][m]] = vals[order][m]
        oh[p_idx[m], t_idx[m], P + r128[order][m]] = 1.0
        wt = np.ascontiguousarray(
            w[c * O_SHARD : (c + 1) * O_SHARD, :].T
        ).astype(ml_dtypes.bfloat16)
        bias_rep = np.broadcast_to(
            bias_np[c * O_SHARD : (c + 1) * O_SHARD], (P, O_SHARD)
        ).copy()
        in_maps.append({"xt": xt, "wt": wt, "bias": bias_rep, "oh": oh})
    return bucket_tiles, in_maps


def kernel(x, weight, bias, shira_weight, shira_indices, _trace=False):
    bucket_tiles, in_maps = _prep_inputs(
        x, weight, bias, shira_weight, shira_indices
    )
    nc = _build_bass(bucket_tiles)
    _split_multi_waits(nc)
    res = run_bass_kernel_spmd(
        nc, in_maps, core_ids=list(range(N_CORES)), trace=_trace
    )
    out = np.concatenate([r["out"] for r in res.results], axis=1)
    out = out.reshape(4, 2048, OUT_F)
    if _trace:
        kernel.last_results = res
    return out


# revision 12
# speedup vs baseline: 1.5023x; 1.0297x over previous
"""Trainium2 kernel for nn_Linear_14912126452257 (scatter_memory).

Computes: new_weight = weight + scatter_add(shira_indices, shira_weight);
          out = x @ new_weight^T + bias

Sharding: column-parallel over out_features across 8 NeuronCores
(each core owns 512 of 4096 output features). x is replicated; the
sparse COO entries are partitioned by owning row-shard.

Per-core device algorithm:
  1. Scatter: COO entries, bucketed by (c//128, r_loc//128) and padded
     to 128-entry tiles, arrive as host-marshalled one-hot tile pairs in
     fp8-e3m4 (vcoh[j, c%128] = v*64, roh[j, r%128] = 1).  The PE
     accumulates delta^T[c, o] = vcoh^T @ roh into PSUM (duplicates add
     natively); DVE then fuses descale+add: W'^T = pd/64 + W^T, bf16.
  2. GEMM: out[m, o] = sum_ic xT[ic]^T @ W'^T[ic] in bf16 with fp32
     PSUM accumulation, + bias epilogue on DVE.  The first six GEMM
     m-tiles are interleaved into the scatter loop (accumulating chunk
     by chunk as W' chunks appear) so the PE stays busy while the
     one-hot tiles stream in from HBM.
Host only marshals data (transpose/cast/bucket/pad/one-hot expand) and
concatenates the per-core output shards.
"""

import sys

for _p in ("/opt/trn_rl_repo", "/root/.axon_site/_ro/trn_rl_repo"):
    if _p not in sys.path:
        sys.path.append(_p)

import numpy as np
import ml_dtypes

import concourse.bass as bass
import concourse.mybir as mybir
import concourse.tile as tile
from concourse.bass_utils import run_bass_kernel_spmd

P = 128
IN_F = 4096
OUT_F = 4096
N_CORES = 8
O_SHARD = OUT_F // N_CORES  # 512
NK = IN_F // P  # 32 contraction chunks
NOC = O_SHARD // P  # 4 output sub-chunks per core
M_TOT = 8192  # 4 * 2048 tokens
SUPER_M = 512  # tokens per x super-tile
NSUP = M_TOT // SUPER_M
MT_PER_SUP = SUPER_M // P
SCALING = 1.0
VSCALE = 64.0  # fp8-e3m4 value pre-scale (min normal 2^-2; v ~ 0.02)
N_EARLY = 6  # GEMM m-tiles interleaved into the scatter loop


def _build_bass(bucket_tiles):
    """Build the SPMD Bass program. bucket_tiles[ic][oc] = number of
    128-entry one-hot tile pairs for bucket (ic, oc); same for every
    core (padded)."""
    ic_tiles = [int(sum(bucket_tiles[ic])) for ic in range(NK)]
    t_total = int(sum(ic_tiles))
    nc = bass.Bass("TRN2", target_bir_lowering=False, debug=False, num_devices=1)

    xt_d = nc.dram_tensor("xt", [IN_F, M_TOT], mybir.dt.bfloat16, kind="ExternalInput").ap()
    wt_d = nc.dram_tensor("wt", [IN_F, O_SHARD], mybir.dt.bfloat16, kind="ExternalInput").ap()
    bias_d = nc.dram_tensor("bias", [P, O_SHARD], mybir.dt.float32, kind="ExternalInput").ap()
    # one-hot tile pairs: [:, t, 0:128] = vcoh (values*64), [:, t, 128:256] = roh
    oh_d = nc.dram_tensor("oh", [P, t_total, 2 * P], mybir.dt.float8e3, kind="ExternalInput").ap()
    out_d = nc.dram_tensor("out", [M_TOT, O_SHARD], mybir.dt.float32, kind="ExternalOutput").ap()

    xt_t = xt_d.rearrange("(ko p) m -> p ko m", p=P)  # [P, NK, M_TOT]
    out_t = out_d.rearrange("(mt p) o -> mt p o", p=P)

    with tile.TileContext(nc) as tc:
        with (
            tc.tile_pool(name="persist", bufs=1) as persist,
            tc.tile_pool(name="work", bufs=3) as work,
            tc.tile_pool(name="ohpool", bufs=2) as ohpool,
            tc.tile_pool(name="xpool", bufs=3) as xpool,
        ):
            wt_bf = persist.tile([P, NK, O_SHARD], mybir.dt.bfloat16)
            bias_sb = persist.tile([P, O_SHARD], mybir.dt.float32)
            nc.sync.dma_start(bias_sb[:], bias_d[:])

            def load_sup(sup, defer=False):
                xsb = xpool.tile([P, NK, SUPER_M], mybir.dt.bfloat16, tag="xsb")
                if not defer:
                    nc.sync.dma_start(
                        xsb[:], xt_t[:, :, sup * SUPER_M : (sup + 1) * SUPER_M]
                    )
                return xsb

            def load_sup_chunk(xsb, sup, ic):
                nc.sync.dma_start(
                    xsb[:, ic, :],
                    xt_t[:, ic, sup * SUPER_M : (sup + 1) * SUPER_M],
                )

            def epilogue(po, sup, mt):
                osb = work.tile([P, O_SHARD], mybir.dt.float32, tag="osb")
                nc.vector.tensor_tensor(
                    out=osb[:], in0=po[:], in1=bias_sb[:], op=mybir.AluOpType.add
                )
                nc.sync.dma_start(out_t[sup * MT_PER_SUP + mt], osb[:])

            xsb0 = load_sup(0, defer=True)
            xsb1 = load_sup(1, defer=True)
            early_src = [
                (xsb0, 0, 0), (xsb0, 0, 1), (xsb0, 0, 2), (xsb0, 0, 3),
                (xsb1, 1, 0), (xsb1, 1, 1),
            ][:N_EARLY]

            # ---- opening: scatter W' chunks, with early GEMM interleaved ----
            # single PSUM pool for the whole kernel: tags pd(x2) + early0..5
            # cover all 8 banks; the main GEMM rotates over the same tags so
            # bank reuse carries exact deps (no pool-release barriers).
            with tc.tile_pool(name="psum", bufs=1, space="PSUM") as psum_pool:
                early = [
                    psum_pool.tile([P, O_SHARD], mybir.dt.float32, name=f"early{k}", tag=f"early{k}")
                    for k in range(N_EARLY)
                ]
                def early_mms(ic):
                    # early GEMM matmuls for chunk ic (lagged one chunk so
                    # the DVE chunk-assembly overlaps PE work)
                    for k, (xsb, _, mt) in enumerate(early_src):
                        nc.tensor.matmul(
                            out=early[k][:],
                            lhsT=xsb[:, ic, mt * P : (mt + 1) * P],
                            rhs=wt_bf[:, ic, :],
                            start=(ic == 0), stop=(ic == NK - 1),
                        )

                tbase = 0
                for ic in range(NK):
                    nt = ic_tiles[ic]
                    ohc = ohpool.tile([P, nt, 2 * P], mybir.dt.float8e3, tag="ohc")
                    nc.sync.dma_start(ohc[:], oh_d[:, tbase : tbase + nt, :])
                    wtile = work.tile([P, O_SHARD], mybir.dt.bfloat16, tag="wtile")
                    nc.sync.dma_start(wtile[:], wt_d[ic * P : (ic + 1) * P, :])
                    load_sup_chunk(xsb0, 0, ic)
                    load_sup_chunk(xsb1, 1, ic)
                    pd = psum_pool.tile([P, O_SHARD], mybir.dt.float32, tag="pd", bufs=2)
                    t = 0
                    for oc in range(NOC):
                        ntoc = int(bucket_tiles[ic][oc])
                        for j in range(ntoc):
                            nc.tensor.matmul(
                                out=pd[:, oc * P : (oc + 1) * P],
                                lhsT=ohc[:, t, 0:P],
                                rhs=ohc[:, t, P : 2 * P],
                                start=(j == 0), stop=(j == ntoc - 1),
                            )
                            t += 1
                    tbase += nt
                    # W'^T chunk = pd/VSCALE + W^T chunk, cast bf16
                    nc.vector.scalar_tensor_tensor(
                        out=wt_bf[:, ic, :],
                        in0=pd[:],
                        scalar=1.0 / VSCALE,
                        in1=wtile[:],
                        op0=mybir.AluOpType.mult,
                        op1=mybir.AluOpType.add,
                    )
                    if ic >= 2:
                        early_mms(ic - 2)
                early_mms(NK - 2)
                early_mms(NK - 1)

                # ---- main GEMM: remaining m-tiles ----
                tag_seq = ["pd", "pd"] + [f"early{k}" for k in range(N_EARLY)]
                tile_ctr = [0]

                def gemm_tile(xsb, sup, mt):
                    tag = tag_seq[tile_ctr[0] % len(tag_seq)]
                    tile_ctr[0] += 1
                    po = psum_pool.tile(
                        [P, O_SHARD], mybir.dt.float32, name="po", tag=tag,
                        bufs=2 if tag == "pd" else 1,
                    )
                    for ic in range(NK):
                        nc.tensor.matmul(
                            out=po[:],
                            lhsT=xsb[:, ic, mt * P : (mt + 1) * P],
                            rhs=wt_bf[:, ic, :],
                            start=(ic == 0), stop=(ic == NK - 1),
                        )
                    epilogue(po, sup, mt)

                # interleave the early-tile epilogues between the first main
                # tiles: main tiles grab the freed psum_d banks first, so the
                # PE never waits on the epilogue chain at the phase switch
                done = {(s, m) for (_, s, m) in early_src}
                rest1 = [mt for mt in range(MT_PER_SUP) if (1, mt) not in done]
                gemm_tile(xsb1, 1, rest1[0])
                for k in range(3):
                    epilogue(early[k], early_src[k][1], early_src[k][2])
                gemm_tile(xsb1, 1, rest1[1])
                for k in range(3, N_EARLY):
                    epilogue(early[k], early_src[k][1], early_src[k][2])
                for sup in range(2, NSUP):
                    xsb = load_sup(sup)
                    for mt in range(MT_PER_SUP):
                        gemm_tile(xsb, sup, mt)
    return nc


def _split_multi_waits(nc):
    """Walrus in this container rejects compute-engine instructions carrying
    more than one sync wait (setupSyncWait: 'Too many sync wait commands').
    Hoist all-but-none of each such instruction's waits onto standalone
    EventSemaphore (pure wait) instructions inserted just before it in the
    same engine stream — semantically identical, per-engine order preserved."""
    import concourse.mybir as mybir

    n_split = 0
    for fn in nc.m.functions:
        for block in fn.blocks:
            new_instructions = []
            for inst in block.instructions:
                si = getattr(inst, "sync_info", None)
                waits = list(si.on_wait) if si is not None else []
                if len(waits) > 1:
                    for w in waits:
                        n_split += 1
                        new_instructions.append(
                            mybir.InstEventSemaphore(
                                name=f"{inst.name}-w{n_split}",
                                engine=inst.engine,
                                ins=[],
                                outs=[],
                                sync_info=mybir.SyncInfo(
                                    on_wait=[w], on_update=[]
                                ),
                            )
                        )
                    inst.sync_info = mybir.SyncInfo(
                        on_wait=[], on_update=list(si.on_update)
                    )
                new_instructions.append(inst)
            block.instructions = new_instructions
    return n_split


def _prep_inputs(x, weight, bias, shira_weight, shira_indices):
    """Host-side marshalling: transpose/cast x, shard+transpose W, bucket
    the COO entries by (core, c//128, r_loc//128), pad to 128-entry tiles
    and expand into fp8 one-hot tile pairs."""
    x2 = np.asarray(x, dtype=np.float32).reshape(M_TOT, IN_F)
    xt = np.ascontiguousarray(x2.T).astype(ml_dtypes.bfloat16)

    w = np.asarray(weight, dtype=np.float32)
    bias_np = np.asarray(bias, dtype=np.float32)
    rows = np.asarray(shira_indices[0]).astype(np.int64)
    cols = np.asarray(shira_indices[1]).astype(np.int64)
    vals = (np.asarray(shira_weight, dtype=np.float32) * SCALING * VSCALE).astype(
        ml_dtypes.float8_e3m4
    )

    core = rows // O_SHARD
    r_loc = rows % O_SHARD
    oc = r_loc // P
    r128 = r_loc % P
    ic = cols // P
    c128 = cols % P

    # bucket = (core, ic, oc); counts per bucket
    NB = NK * NOC
    bucket = ic * NOC + oc  # 0..127 within a core
    counts = np.zeros((N_CORES, NB), dtype=np.int64)
    np.add.at(counts, (core, bucket), 1)
    # padded tiles per bucket: max across cores, at least 1
    bt_flat = np.maximum(1, -(-counts.max(axis=0) // P))  # [NB]
    bucket_tiles = bt_flat.reshape(NK, NOC)
    t_total = int(bt_flat.sum())
    tile_base = np.concatenate([[0], np.cumsum(bt_flat)[:-1]])  # [NB]

    # sort entries by (core, bucket); rank within segment
    key = core * NB + bucket
    order = np.argsort(key, kind="stable")
    key_s = key[order]
    seg_starts = np.searchsorted(key_s, np.arange(N_CORES * NB))
    rank = np.arange(len(order)) - np.repeat(
        seg_starts, np.diff(np.concatenate([seg_starts, [len(order)]]))
    )
    core_s = core[order]
    b_s = bucket[order]
    # global entry slot: tile t = tile_base[b] + rank//P, row p = rank%P
    t_idx = tile_base[b_s] + rank // P
    p_idx = rank % P

    in_maps = []
    for c in range(N_CORES):
        m = core_s == c
        # oh[p, t, 0:128] = vcoh (value one-hot over c128, pre-scaled)
        # oh[p, t, 128:256] = roh (one-hot over r128)
        oh = np.zeros((P, t_total, 2 * P), dtype=ml_dtypes.float8_e3m4)
        oh[p_idx[m], t_idx[m], c128[order][m]] = vals[order][m]
        oh[p_idx[m], t_idx[m], P + r128[order][m]] = 1.0
        wt = np.ascontiguousarray(
            w[c * O_SHARD : (c + 1) * O_SHARD, :].T
        ).astype(ml_dtypes.bfloat16)
        bias_rep = np.broadcast_to(
            bias_np[c * O_SHARD : (c + 1) * O_SHARD], (P, O_SHARD)
        ).copy()
        in_maps.append({"xt": xt, "wt": wt, "bias": bias_rep, "oh": oh})
    return bucket_tiles, in_maps


def kernel(x, weight, bias, shira_weight, shira_indices, _trace=False):
    bucket_tiles, in_maps = _prep_inputs(
        x, weight, bias, shira_weight, shira_indices
    )
    nc = _build_bass(bucket_tiles)
    _split_multi_waits(nc)
    res = run_bass_kernel_spmd(
        nc, in_maps, core_ids=list(range(N_CORES)), trace=_trace
    )
    out = np.concatenate([r["out"] for r in res.results], axis=1)
    out = out.reshape(4, 2048, OUT_F)
    if _trace:
        kernel.last_results = res
    return out
